# revision 17
# baseline (speedup 1.0000x reference)
"""Causal single-head attention (B=4, T=4096, D=1024, H=64) on 8 TRN2 cores.

Sharding: 2 cores per batch; queries split so both cores get one tile per
size class (balanced 72 causal chunks each):
  half0 (cores 0-3): query tiles {0,3,4,7}   half1 (cores 4-7): {1,2,5,6}

BRANCH-FREE UNIFORM PROGRAM: both halves execute the identical
instruction stream; all per-half differences live in host-prepared data:
  - x^T (bf16, [D,T]) is column-PERMUTED per core (tile order pi) so the
    core's own query tiles sit at even 512-positions 0,2,4,6 and every
    slot's causal key prefix occupies positional chunks [0, 8r+8).
  - slot r processes positional chunks 0..8r+7 (8,16,24,32 = 80 chunks,
    8 of them zero-masked padding), applying data-driven masks to its
    last 8 chunks.  Masks are built on-device from a host ramp
    ((t'-p)/4, exact in bf16) compared against a per-core sigma table:
    sigma=-big -> all-ones (real past), +big -> all-zero (pad/future),
    128j/4 -> causal staircase for the slot's diagonal chunks.
Projections stream with d on partitions (no on-device transpose of x);
k-hi / q-lo partition copies are per-pair SBUF->SBUF DMAs on the sync
queue so score matmuls run row-packed in concurrent pairs; v^T returns
to natural [t,h] via small PE transposes into vsb with a ones column
(PV matmul m=65 also yields the softmax denominator).  Projection pairs
and attention slots interleave freely (no If barriers), so Exp overlaps
the x DMA stream.  Softmax runs without max-subtraction.
"""

import numpy as np
import ml_dtypes

import concourse.bass as bass
import concourse.mybir as mybir
from concourse import bacc
from concourse.tile import TileContext
from concourse.masks import make_identity
from concourse.bass_utils import run_bass_kernel_spmd

B, T, D, H = 4, 4096, 1024, 64
NCORES = 8
NQ = 2048
SCALE = 1.0 / np.sqrt(D)  # 1/32
BF16 = ml_dtypes.bfloat16
BIG = 8192.0

HALF_TILES = {0: [0, 3, 4, 7], 1: [1, 2, 5, 6]}
# position -> original tile (self-inverse swaps)
POS_TILES = {0: [0, 1, 3, 2, 4, 5, 7, 6], 1: [1, 0, 2, 3, 5, 4, 6, 7]}

_CACHE = {}


def _sigma(half):
    # sigma/4 for mask (r, j): slot r's positional chunk 8r+j, j in 0..7
    sig = np.zeros(32, np.float32)
    for r in range(4):
        tau = HALF_TILES[half][r]
        for j in range(8):
            pt = POS_TILES[half][2 * r + j // 4]
            if pt < tau:
                sig[8 * r + j] = -BIG
            elif pt > tau:
                sig[8 * r + j] = BIG
            else:
                sig[8 * r + j] = 128.0 * (j % 4) / 4.0
    return sig


def _build():
    if "nc" in _CACHE:
        return _CACHE["nc"]
    f32 = mybir.dt.float32
    bf16 = mybir.dt.bfloat16
    AF = mybir.ActivationFunctionType

    nc = bacc.Bacc(None, target_bir_lowering=False)
    xt_d = nc.declare_dram_parameter("xt", [D, T], bf16, isOutput=False)
    wkq_d = nc.declare_dram_parameter("wkq", [D, 128], bf16, isOutput=False)
    wv_d = nc.declare_dram_parameter("wv", [D, H], bf16, isOutput=False)
    rmp_d = nc.declare_dram_parameter("ramp", [128, 512], bf16, isOutput=False)
    sig_d = nc.declare_dram_parameter("sig", [128, 32], f32, isOutput=False)
    out_d = nc.declare_dram_parameter("out", [NQ, H], f32, isOutput=True)
    outd_v = out_d[:, :].rearrange("(s c p) h -> s p c h", p=128, c=4)

    with TileContext(nc) as tc:
        with (
            tc.tile_pool(name="persist", bufs=1) as pp,
            tc.tile_pool(name="work", bufs=2) as pw,
            tc.tile_pool(name="ps1", bufs=1, space="PSUM") as ps1,
            tc.tile_pool(name="ps2", bufs=1, space="PSUM") as ps2,
        ):
            # small inputs first on the gpsimd DMA queue
            wkq = pp.tile([128, 1024], bf16, tag="wkq")
            nc.gpsimd.dma_start(
                out=wkq[:, :].rearrange("p (c h) -> p c h", h=128),
                in_=wkq_d[:, :].rearrange("(c p) h -> p c h", p=128))
            wv = pp.tile([128, 512], bf16, tag="wv")
            nc.gpsimd.dma_start(
                out=wv[:, :].rearrange("p (c h) -> p c h", h=64),
                in_=wv_d[:, :].rearrange("(c p) h -> p c h", p=128))
            ramp = pp.tile([128, 512], bf16, tag="ramp")
            nc.gpsimd.dma_start(out=ramp[:, :], in_=rmp_d[:, :])
            sig = pp.tile([128, 32], f32, tag="sig")
            nc.gpsimd.dma_start(out=sig[:, :], in_=sig_d[:, :])

            # xT in SBUF, one tile per 512-col position group
            xtd_v = xt_d[:, :].rearrange("(c p) t -> p c t", p=128)
            xgs = []
            for g in range(8):
                xg = pp.tile([128, 8 * 512], bf16, tag=f"xg{g}", name=f"xg{g}")
                nc.gpsimd.dma_start(
                    out=xg[:, :].rearrange("p (c t) -> p c t", t=512),
                    in_=xtd_v[:, :, 512 * g: 512 * (g + 1)])
                xgs.append(xg)

            # ---- constants ----
            ident_f = pp.tile([128, 128], f32, tag="idf")
            make_identity(nc, ident_f[:, :])
            ident_b = pp.tile([128, 128], bf16, tag="idb")
            nc.vector.tensor_copy(ident_b[:, :], ident_f[:, :])

            # data-driven masks: maskbuf[:, 512m:512(m+1)] = (ramp >= sig[m])
            maskbuf = pp.tile([128, 32 * 512], bf16, tag="maskbuf")
            for m in range(32):
                nc.vector.tensor_scalar(
                    out=maskbuf[:, 512 * m: 512 * (m + 1)],
                    in0=ramp[:, :], scalar1=sig[:, m: m + 1], scalar2=None,
                    op0=mybir.AluOpType.is_ge)

            # preload the exp activation table off the critical path
            warm = pp.tile([1, 2], f32, tag="warm")
            nc.vector.memset(warm[:, 0:1], 0.0)
            nc.scalar.activation(warm[:, 1:2], warm[:, 0:1], AF.Exp)

            # persistent per-pair activations (pair p = positions 2p, 2p+1)
            kTs = [pp.tile([128, 1024], bf16, tag=f"kT{p}", name=f"kT{p}") for p in range(4)]
            qTs = [pp.tile([64, 512], bf16, tag=f"qT{p}", name=f"qT{p}") for p in range(4)]
            qkg = [pp.tile([128, 512], bf16, tag=f"qkg{g}", name=f"qkg{g}") for g in range(8)]
            vsbs = []
            for p in range(4):
                vs = pp.tile([128, 8 * 65], bf16, tag=f"vsb{p}", name=f"vsb{p}")
                nc.vector.memset(vs[:, :], 1.0)  # col 64 of each chunk = 1
                vsbs.append(vs)

            ps_bufs = [ps2.tile([128, 1024], f32, tag=f"sc{i}", name=f"scb{i}")
                       for i in range(2)]
            pT_bufs = [pw.tile([128, 1024], bf16, tag=f"pT{i}", name=f"pTb{i}")
                       for i in range(2)]
            po_bufs = [ps2.tile([65, 512], f32, tag=f"po{i}", name=f"pob{i}")
                       for i in range(2)]
            osb = pw.tile([65, 512], f32, tag="osb")
            rc = pw.tile([128, 4], f32, tag="rc")
            outsbs = [pw.tile([128, 256], f32, tag=f"osl{s}", name=f"oslb{s}")
                      for s in range(4)]

            state = {"gi": 0}

            def project(g):
                xg = xgs[g]
                p, gp = g // 2, g % 2
                pqk = ps1.tile([128, 512], f32, tag="qk")
                for dc in range(8):
                    nc.tensor.matmul(
                        pqk[:, :], lhsT=wkq[:, 128 * dc: 128 * (dc + 1)],
                        rhs=xg[:, 512 * dc: 512 * (dc + 1)],
                        start=(dc == 0), stop=(dc == 7))
                nc.vector.tensor_copy(qkg[g][:, :], pqk[:, :])
                if gp == 0:
                    # q of the even position is all attention needs: shift it
                    # to partitions 0:64 as soon as it lands
                    nc.sync.dma_start(out=qTs[p][:, :], in_=qkg[g][64:128, :])

                pv = ps1.tile([64, 512], f32, tag="v")
                for dc in range(8):
                    nc.tensor.matmul(
                        pv[:, :], lhsT=wv[:, 64 * dc: 64 * (dc + 1)],
                        rhs=xg[:, 512 * dc: 512 * (dc + 1)],
                        start=(dc == 0), stop=(dc == 7))
                vT = pw.tile([64, 512], bf16, tag="vT")
                nc.vector.tensor_copy(vT[:, :], pv[:, :])
                pvn = ps_bufs[state["gi"] % 2]
                state["gi"] += 1
                for c in range(4):
                    nc.tensor.matmul(
                        pvn[:, 64 * c: 64 * (c + 1)],
                        lhsT=vT[0:64, 128 * c: 128 * (c + 1)],
                        rhs=ident_b[0:64, 0:64], start=True, stop=True)
                nc.vector.tensor_copy(
                    vsbs[p][:, 65 * 4 * gp: 65 * 4 * (gp + 1)].rearrange(
                        "p (c h) -> p c h", h=65)[:, :, 0:64],
                    pvn[:, 0:256].rearrange("p (c h) -> p c h", h=64))
                if gp == 1:
                    # k duplicated onto partitions 64:128 for row packing
                    nc.sync.dma_start(out=kTs[p][64:128, 0:512], in_=qkg[2 * p][0:64, :])
                    nc.sync.dma_start(out=kTs[p][64:128, 512:1024], in_=qkg[g][0:64, :])

            def kchunk(c, hi):
                if hi == 0:
                    csl = slice(128 * (c % 4), 128 * (c % 4) + 128)
                    return qkg[c // 4][0:64, csl]
                csl = slice(128 * (c % 8), 128 * (c % 8) + 128)
                return kTs[c // 8][64:128, csl]

            def attend(r):
                # slot r: query position 2r, positional chunks 0..8r+7
                n = 8 * r + 8
                qlo = qTs[r][0:64, :]
                qhi = qkg[2 * r][64:128, :]
                po = po_bufs[r % 2]
                for pos in range(0, n, 2):
                    c0, c1 = pos, pos + 1
                    ps = ps_bufs[state["gi"] % 2]
                    pT = pT_bufs[state["gi"] % 2]
                    state["gi"] += 1
                    nc.tensor.matmul(ps[:, 0:512], lhsT=kchunk(c0, 0),
                                     rhs=qlo, start=True, stop=True)
                    nc.tensor.matmul(ps[:, 512:1024], lhsT=kchunk(c1, 1),
                                     rhs=qhi, start=True, stop=True)
                    nc.scalar.activation(pT[:, :], ps[:, :], AF.Exp, scale=SCALE)
                    for jj, ch in enumerate((c0, c1)):
                        if ch >= 8 * r:  # slot's own pair: data-driven mask
                            m = 8 * r + (ch - 8 * r)
                            nc.vector.tensor_mul(
                                pT[:, 512 * jj: 512 * (jj + 1)],
                                pT[:, 512 * jj: 512 * (jj + 1)],
                                maskbuf[:, 512 * m: 512 * (m + 1)])
                        nc.tensor.matmul(
                            po[:, :],
                            lhsT=vsbs[ch // 8][:, 65 * (ch % 8): 65 * (ch % 8) + 65],
                            rhs=pT[:, 512 * jj: 512 * (jj + 1)],
                            start=(ch == 0), stop=(ch == n - 1))

                # epilogue: transpose [65,512] -> [512,65], divide, store slot
                nc.vector.tensor_copy(osb[:, :], po[:, :])
                pe2 = ps_bufs[state["gi"] % 2]
                state["gi"] += 1
                for c in range(4):
                    nc.tensor.matmul(
                        pe2[:, 65 * c: 65 * (c + 1)],
                        lhsT=osb[0:65, 128 * c: 128 * (c + 1)],
                        rhs=ident_f[0:65, 0:65], start=True, stop=True)
                outsb = outsbs[r]
                for c in range(4):
                    nc.vector.reciprocal(rc[:, c: c + 1], pe2[:, 65 * c + 64: 65 * c + 65])
                    nc.vector.tensor_scalar_mul(
                        outsb[:, 64 * c: 64 * (c + 1)],
                        pe2[:, 65 * c: 65 * c + 64], rc[:, c: c + 1])
                nc.sync.dma_start(
                    out=outd_v[r, :, :, :],
                    in_=outsb[:, :].rearrange("p (c h) -> p c h", h=64))

            for r in range(4):
                project(2 * r)
                project(2 * r + 1)
                attend(r)

    nc.compile()
    _CACHE["nc"] = nc
    return nc


def _in_maps(x, Wq, Wk, Wv):
    wkq = np.concatenate([Wk, Wq], axis=1).astype(BF16)  # [D, 128], k first
    wv = np.asarray(Wv).astype(BF16)
    ramp = ((np.arange(512)[None, :] - np.arange(128)[:, None]) / 4.0).astype(BF16)
    ramp = np.ascontiguousarray(ramp)
    sigs = {h: np.ascontiguousarray(
        np.broadcast_to(_sigma(h)[None, :], (128, 32)).astype(np.float32))
        for h in (0, 1)}
    xts = {}
    for b in range(B):
        xt = np.asarray(x[b], np.float32).T.astype(BF16)  # [D, T]
        for h in (0, 1):
            xp = np.empty_like(xt)
            for pos in range(8):
                t = POS_TILES[h][pos]
                xp[:, 512 * pos: 512 * (pos + 1)] = xt[:, 512 * t: 512 * (t + 1)]
            xts[(b, h)] = np.ascontiguousarray(xp)
    maps = []
    for c in range(NCORES):
        b, h = c % 4, c // 4
        maps.append({"xt": xts[(b, h)], "wkq": wkq, "wv": wv,
                     "ramp": ramp, "sig": sigs[h]})
    return maps


def _install_profile_shim():
    import sys, types
    import concourse.bass_utils as bu
    bu.upload_artifacts = lambda tmpdir: "local://" + tmpdir
    if "antenv.axon_hooks" in sys.modules:
        return
    mod = types.ModuleType("antenv.axon_hooks")
    holder = []
    mod.set_axon_ntff_profile_hook = holder.append
    mod.get_axon_ntff_profile_hook = lambda: holder[-1] if holder else None
    sys.modules["antenv.axon_hooks"] = mod
    import antenv
    antenv.axon_hooks = mod
    from trn_agent_boot.trn_boot import _ntff_profile_via_ctypes
    mod.set_axon_ntff_profile_hook(_ntff_profile_via_ctypes("/opt/axon/libaxon_pjrt.so"))


def kernel(x, Wq, Wk, Wv, _want_profile=False):
    if _want_profile:
        _install_profile_shim()
    nc = _build()
    maps = _in_maps(x, Wq, Wk, Wv)
    res = run_bass_kernel_spmd(nc, maps, core_ids=list(range(NCORES)),
                               trace=_want_profile)
    out = np.empty((B, T, H), np.float32)
    for c in range(NCORES):
        b, half = c % 4, c // 4
        r = np.asarray(res.results[c]["out"])
        for slot, t in enumerate(HALF_TILES[half]):
            out[b, 512 * t: 512 * (t + 1)] = r[512 * slot: 512 * (slot + 1)]
    if _want_profile:
        return out, res
    return out
